# revision 9
# baseline (speedup 1.0000x reference)
"""GCN 3-layer classifier on 8 Trainium2 NeuronCores.

Strategy: partition dst nodes (and incident edges) across the 8 cores.
Each core:
  P0: computes hpre1' = (x @ W1) * dinv for ALL nodes (replicated; cheap on PE)
      stored as bf16 rows [Npad, 128] in its own DRAM.
  L1: message passing for its dst shard: edges grouped by (8-block
      super-block, src-range); dma_gather of source rows (int16 idx per
      25088-row range table), one-hot built on DVE (is_equal vs iota),
      segment-sum via PSUM-accumulated matmuls; per dst-block epilogue
      applies dinv/bias/relu and the W2 matmul, producing hpre2' rows.
  AllGather (4 range-chunks, overlapped) -> full hpre2' table per core.
  L2: same message passing; epilogue pools per-graph sums via one-hot
      matmul into a persistent PSUM accumulator.
  AllReduce pooled sums + on-device MLP -> [64, 10] logits.

Self-loops are extra edges (src=dst); the GCN normalization
norm_e = dinv[src]*dinv[dst] factorizes: dinv[src] is folded into the
gathered rows (hpre' pre-scaled), dinv[dst] applied per dst block.
"""

import sys

for _p in ("/opt/trn_rl_repo", "/root/.axon_site/_ro/trn_rl_repo"):
    if _p not in sys.path:
        sys.path.append(_p)

import numpy as np
import ml_dtypes

N = 100000
E = 1600000
G = 64
IN_DIM = 64
HID = 128
NCLS = 10

NCORES = 8
SH = 12544            # nodes per core shard (98 blocks of 128)
NPAD = SH * NCORES    # 100352
NB = 98               # dst blocks per core
BPS = 8               # blocks per super-block
NSB = 13              # super-blocks (12*8 + 2)
R = 4                 # src ranges (int16 gather tables < 32768 rows)
R1 = NPAD // R        # 25088: layer-1 range size (global contiguous)
R2 = SH // R          # 3136: layer-2 per-core slice size

BF16 = ml_dtypes.bfloat16

_CACHE = {}


def _blocks_of(sb):
    return list(range(sb * BPS, min(sb * BPS + BPS, NB)))


def _build_layer_schedule(all_src, all_dst, r_of_src, idx_of_src):
    """Shared chunk schedule + per-core padded slot arrays for one layer."""
    core = all_dst // SH
    b = (all_dst % SH) // 128
    dstloc = all_dst % 128
    r = r_of_src
    key = (core.astype(np.int64) * NB + b) * R + r
    counts = np.bincount(key, minlength=NCORES * NB * R).reshape(NCORES, NB, R)
    nch = np.maximum(0, -(-counts.max(axis=0) // 128))  # [NB, R] ceil/128 of max core

    # schedule order: (sb, r, b in sb)
    order = []
    for sb in range(NSB):
        for rr in range(R):
            for bb in _blocks_of(sb):
                order.append((bb, rr))
    ordpos = np.zeros((NB, R), np.int64)
    for i, (bb, rr) in enumerate(order):
        ordpos[bb, rr] = i
    nch_ord = np.array([nch[bb, rr] for (bb, rr) in order], np.int64)
    chunk_start_ord = np.concatenate([[0], np.cumsum(nch_ord)])[:-1]
    total_chunks = int(nch_ord.sum())
    slot_tot = total_chunks * 128

    group_slot_off = chunk_start_ord * 128  # per schedule-order group

    idx_tensors, dst_tensors = [], []
    for c in range(NCORES):
        sel = core == c
        k2 = ordpos[b[sel], r[sel]]
        perm = np.argsort(k2, kind="stable")
        k2s = k2[perm]
        grp_first = np.searchsorted(k2s, np.arange(len(order)))
        within = np.arange(len(k2s)) - grp_first[k2s]
        pos = group_slot_off[k2s] + within
        assert pos.max(initial=-1) < slot_tot

        idx_pad = np.zeros(slot_tot, np.int16)
        dst_pad = np.full(slot_tot, 200.0, np.float32)
        idx_pad[pos] = idx_of_src[sel][perm].astype(np.int16)
        dst_pad[pos] = dstloc[sel][perm]

        # wrapped int16 layout: slot i -> [16*g + i%16, i//16], replicated 8x
        wrapped = np.tile(idx_pad.reshape(-1, 16).T, (8, 1))
        idx_tensors.append(np.ascontiguousarray(wrapped))
        # dst layout: chunk j col j, partition = slot%128; bf16
        dst_tensors.append(np.ascontiguousarray(dst_pad.reshape(-1, 128).T.astype(BF16)))

    # per-block first/last chunk flags (r, k)
    blk_first, blk_last = {}, {}
    for bb in range(NB):
        rs = [rr for rr in range(R) if nch[bb, rr] > 0]
        assert rs, f"block {bb} has no chunks"
        blk_first[bb] = (rs[0], 0)
        blk_last[bb] = (rs[-1], int(nch[bb, rs[-1]]) - 1)

    return {
        "nch": nch,
        "total_chunks": total_chunks,
        "slot_tot": slot_tot,
        "idx": idx_tensors,
        "dst": dst_tensors,
        "blk_first": blk_first,
        "blk_last": blk_last,
    }


def _preprocess(x, src, dst, batch, W1, b1, W2, b2, Wl1, bl1, Wl2, bl2):
    src = np.asarray(src, np.int64)
    dst = np.asarray(dst, np.int64)
    batch = np.asarray(batch, np.int64)

    deg = np.bincount(dst, minlength=N).astype(np.float32) + 1.0
    dinv = 1.0 / np.sqrt(deg)
    dinv_pad = np.zeros(NPAD, np.float32)
    dinv_pad[:N] = dinv

    self_n = np.arange(N, dtype=np.int64)
    all_src = np.concatenate([src, self_n])
    all_dst = np.concatenate([dst, self_n])

    # layer 1 table: hpre1 rows, ranges = global contiguous quarters
    sched1 = _build_layer_schedule(all_src, all_dst, all_src // R1,
                                   all_src % R1)
    # layer 2 table: allgathered (core-major, shard-quarter) order
    c_of = all_src // SH
    l_of = all_src % SH
    sched2 = _build_layer_schedule(all_src, all_dst, l_of // R2,
                                   c_of * R2 + (l_of % R2))

    x_pad = np.zeros((NPAD, IN_DIM), np.float32)
    x_pad[:N] = np.asarray(x, np.float32)
    xT = np.ascontiguousarray(x_pad.T.astype(BF16))  # [64, NPAD]

    cnts = np.bincount(batch, minlength=G).astype(np.float32)
    invcnt = (1.0 / np.maximum(cnts, 1.0)).reshape(G, 1).astype(np.float32)

    common = {
        "xT": xT,
        "W1": np.asarray(W1, np.float32).astype(BF16),                # [64, 128]
        "W2": np.ascontiguousarray(np.asarray(W2, np.float32)),       # [128, 128]
        "Wl1": np.ascontiguousarray(np.asarray(Wl1, np.float32)),     # [128, 64]
        "Wl2": np.ascontiguousarray(np.asarray(Wl2, np.float32)),     # [64, 10]
        "b1b": np.tile(np.asarray(b1, np.float32)[None, :], (128, 1)),
        "b2b": np.tile(np.asarray(b2, np.float32)[None, :], (128, 1)),
        "bl1b": np.tile(np.asarray(bl1, np.float32)[None, :], (G, 1)),
        "bl2b": np.tile(np.asarray(bl2, np.float32)[None, :], (G, 1)),
        "dinv4": np.ascontiguousarray(dinv_pad.reshape(NPAD // 128, 128).T),
        "invcnt": invcnt,
    }

    in_maps = []
    for c in range(NCORES):
        lo = c * SH
        dinvb = np.ascontiguousarray(dinv_pad[lo:lo + SH].reshape(NB, 128).T)
        pooloh = np.zeros((SH, G), np.float32)
        hi_real = min(N, lo + SH)
        if hi_real > lo:
            loc = np.arange(hi_real - lo)
            pooloh[loc, batch[lo:hi_real]] = 1.0
        m = dict(common)
        m["dinvb"] = dinvb
        m["pooloh"] = np.ascontiguousarray(pooloh.astype(BF16))
        m["idx1"] = sched1["idx"][c]
        m["dst1"] = sched1["dst"][c]
        m["idx2"] = sched2["idx"][c]
        m["dst2"] = sched2["dst"][c]
        in_maps.append(m)

    return sched1, sched2, in_maps


DEBUG = False


def _build_program(sched1, sched2):
    import concourse.bass as bass
    import concourse.mybir as mybir
    import concourse.tile as tile
    from concourse import bacc
    from concourse.masks import make_identity

    FP32 = mybir.dt.float32
    BF = mybir.dt.bfloat16
    AOP = mybir.AluOpType
    ACTF = mybir.ActivationFunctionType

    nc = bacc.Bacc("TRN2", target_bir_lowering=False, debug=False,
                   num_devices=NCORES)

    # ---- I/O -----------------------------------------------------------
    xT = nc.dram_tensor("xT", [IN_DIM, NPAD], BF, kind="ExternalInput")
    W1 = nc.dram_tensor("W1", [IN_DIM, HID], BF, kind="ExternalInput")
    W2 = nc.dram_tensor("W2", [HID, HID], FP32, kind="ExternalInput")
    Wl1 = nc.dram_tensor("Wl1", [HID, HID // 2], FP32, kind="ExternalInput")
    Wl2 = nc.dram_tensor("Wl2", [HID // 2, NCLS], FP32, kind="ExternalInput")
    b1b = nc.dram_tensor("b1b", [128, HID], FP32, kind="ExternalInput")
    b2b = nc.dram_tensor("b2b", [128, HID], FP32, kind="ExternalInput")
    bl1b = nc.dram_tensor("bl1b", [G, HID // 2], FP32, kind="ExternalInput")
    bl2b = nc.dram_tensor("bl2b", [G, NCLS], FP32, kind="ExternalInput")
    dinv4 = nc.dram_tensor("dinv4", [128, NPAD // 128], FP32, kind="ExternalInput")
    dinvb = nc.dram_tensor("dinvb", [128, NB], FP32, kind="ExternalInput")
    invcnt = nc.dram_tensor("invcnt", [G, 1], FP32, kind="ExternalInput")
    pooloh = nc.dram_tensor("pooloh", [SH, G], BF, kind="ExternalInput")
    idx1 = nc.dram_tensor("idx1", list(sched1["idx"][0].shape), mybir.dt.int16,
                          kind="ExternalInput")
    dst1 = nc.dram_tensor("dst1", list(sched1["dst"][0].shape), BF,
                          kind="ExternalInput")
    idx2 = nc.dram_tensor("idx2", list(sched2["idx"][0].shape), mybir.dt.int16,
                          kind="ExternalInput")
    dst2 = nc.dram_tensor("dst2", list(sched2["dst"][0].shape), BF,
                          kind="ExternalInput")
    out = nc.dram_tensor("out", [G, NCLS], FP32, kind="ExternalOutput")
    if DEBUG:
        dbg_hpre1 = nc.dram_tensor("dbg_hpre1", [NPAD, HID], BF,
                                   kind="ExternalOutput")
        dbg_ccin = nc.dram_tensor("dbg_ccin", [SH, HID], BF,
                                  kind="ExternalOutput")
        dbg_pooled = nc.dram_tensor("dbg_pooled", [G, HID], FP32,
                                    kind="ExternalOutput")

    # ---- internal DRAM -------------------------------------------------
    hpre1 = nc.dram_tensor("hpre1", [NPAD, HID], BF, kind="Internal")
    cc_in = nc.dram_tensor("cc_in", [SH, HID], BF, kind="Internal")
    cc_out = [
        nc.dram_tensor(f"cc_out{r}", [R1, HID], BF, kind="Internal",
                       addr_space="Shared")
        for r in range(R)
    ]
    cc2_in = nc.dram_tensor("cc2_in", [G, HID], FP32, kind="Internal")
    cc2_out = nc.dram_tensor("cc2_out", [G, HID], FP32, kind="Internal",
                             addr_space="Shared")

    max_call_nch = 0
    sb_nch = {}
    for sched in (sched1, sched2):
        for sb in range(NSB):
            blocks = _blocks_of(sb)
            tot = 0
            for r in range(R):
                nchr = int(sum(sched["nch"][b, r] for b in blocks))
                max_call_nch = max(max_call_nch, nchr)
                tot += nchr
            sb_nch[(id(sched), sb)] = tot
    max_sb_nch = max(
        sb_nch[(id(s), sb)] for s in (sched1, sched2) for sb in range(NSB)
    )

    with tile.TileContext(nc) as tc:
        with tc.tile_pool(name="const", bufs=1) as constp:
            w1c = constp.tile([IN_DIM, HID], BF)
            nc.sync.dma_start(out=w1c[:], in_=W1[:])
            w2c = constp.tile([HID, HID], FP32)
            nc.sync.dma_start(out=w2c[:], in_=W2[:])
            wl1c = constp.tile([HID, HID // 2], FP32)
            nc.sync.dma_start(out=wl1c[:], in_=Wl1[:])
            wl2c = constp.tile([HID // 2, NCLS], FP32)
            nc.sync.dma_start(out=wl2c[:], in_=Wl2[:])
            b1c = constp.tile([128, HID], FP32)
            nc.sync.dma_start(out=b1c[:], in_=b1b[:])
            b2c = constp.tile([128, HID], FP32)
            nc.sync.dma_start(out=b2c[:], in_=b2b[:])
            bl1c = constp.tile([G, HID // 2], FP32)
            nc.sync.dma_start(out=bl1c[:], in_=bl1b[:])
            bl2c = constp.tile([G, NCLS], FP32)
            nc.sync.dma_start(out=bl2c[:], in_=bl2b[:])
            dinv4c = constp.tile([128, NPAD // 128], FP32)
            nc.sync.dma_start(out=dinv4c[:], in_=dinv4[:])
            dinvbc = constp.tile([128, NB], FP32)
            nc.sync.dma_start(out=dinvbc[:], in_=dinvb[:])
            invcntc = constp.tile([G, 1], FP32)
            nc.sync.dma_start(out=invcntc[:], in_=invcnt[:])
            ident = constp.tile([128, 128], FP32)
            make_identity(nc, ident[:])
            iota_i = constp.tile([128, 128], mybir.dt.int16)
            nc.gpsimd.iota(iota_i[:], pattern=[[1, 128]], base=0,
                           channel_multiplier=0)
            iota_b = constp.tile([128, 128], BF)
            nc.vector.tensor_copy(out=iota_b[:], in_=iota_i[:])
            zc = constp.tile([128, 512], BF)
            nc.vector.memset(zc[:], 0)

            # ============ P0: hpre1' = (x @ W1) * dinv, all nodes ========
            NCH0 = NPAD // 128  # 784
            with tc.tile_pool(name="p0sb", bufs=3) as p0sb, \
                 tc.tile_pool(name="p0ps", bufs=2, space="PSUM") as p0ps:
                for g0 in range(0, NCH0, 8):
                    xt = p0sb.tile([IN_DIM, 8 * 128], BF, tag="xt")
                    nc.sync.dma_start(out=xt[:],
                                      in_=xT[:, g0 * 128:(g0 + 8) * 128])
                    ps = p0ps.tile([128, 8, HID], FP32, space="PSUM")
                    for j in range(8):
                        nc.tensor.matmul(ps[:, j, :],
                                         xt[:, j * 128:(j + 1) * 128],
                                         w1c[:], start=True, stop=True)
                    stage = p0sb.tile([128, 8, HID], BF, tag="stage")
                    nc.vector.tensor_tensor(
                        out=stage[:], in0=ps[:],
                        in1=dinv4c[:, g0:g0 + 8].unsqueeze(2)
                            .broadcast_to([128, 8, HID]),
                        op=AOP.mult)
                    nc.sync.dma_start(
                        out=hpre1[g0 * 128:(g0 + 8) * 128, :]
                            .rearrange("(j p) f -> p j f", p=128),
                        in_=stage[:])

            # ============ message-passing layers =========================
            def message_layer(layer, sched, idx_t, dst_t, tables, epilogue):
                nch = sched["nch"]
                chunk_global = 0
                for sb in range(NSB):
                    blocks = _blocks_of(sb)
                    sbnch = int(sum(nch[b, r] for b in blocks for r in range(R)))
                    idxt = mp_sb.tile([128, max_sb_nch * 8], mybir.dt.int16,
                                      tag="idxt")
                    nc.sync.dma_start(
                        out=idxt[:, :sbnch * 8],
                        in_=idx_t[:, chunk_global * 8:(chunk_global + sbnch) * 8])
                    dstt = mp_sb.tile([128, max_sb_nch], BF, tag="dstt")
                    nc.sync.dma_start(
                        out=dstt[:, :sbnch],
                        in_=dst_t[:, chunk_global:chunk_global + sbnch])
                    aggps = agg_ps.tile([128, BPS, HID], FP32, space="PSUM")
                    # Zero-fill each PSUM bank with one start=True matmul.
                    # start clears has_written for the WHOLE bank, so the
                    # per-block accumulation groups below (which interleave
                    # within a bank across the range passes) must all use
                    # start=False on a pre-zeroed bank.
                    nc.tensor.matmul(aggps[:, 0:4, :], zc[:, :128], zc[:, :512],
                                     start=True, stop=True, skip_group_check=True)
                    nc.tensor.matmul(aggps[:, 4:8, :], zc[:, :128], zc[:, :512],
                                     start=True, stop=True, skip_group_check=True)
                    ch_in_sb = 0
                    for r in range(R):
                        nchr = int(sum(nch[b, r] for b in blocks))
                        if nchr == 0:
                            continue
                        gt = mp_g.tile([128, max_call_nch, HID], BF, tag="gt")
                        nc.gpsimd.dma_gather(
                            out_ap=gt[:, :nchr, :], in_ap=tables[r],
                            idxs_ap=idxt[:, ch_in_sb * 8:(ch_in_sb + nchr) * 8],
                            num_idxs=nchr * 128, num_idxs_reg=nchr * 128,
                            elem_size=HID, single_packet=False)
                        oht = mp_oh.tile([128, max_call_nch, 128], BF, tag="oht")
                        nc.vector.tensor_tensor(
                            out=oht[:, :nchr, :],
                            in0=iota_b[:].unsqueeze(1)
                                .broadcast_to([128, nchr, 128]),
                            in1=dstt[:, ch_in_sb:ch_in_sb + nchr].unsqueeze(2)
                                .broadcast_to([128, nchr, 128]),
                            op=AOP.is_equal)
                        j = 0
                        for bi, b in enumerate(blocks):
                            for k in range(int(nch[b, r])):
                                nc.tensor.matmul(
                                    aggps[:, bi, :], oht[:, j, :], gt[:, j, :],
                                    start=False,
                                    stop=(sched["blk_last"][b] == (r, k)),
                                    skip_group_check=True)
                                j += 1
                        ch_in_sb += nchr
                    epilogue(sb, blocks, aggps)
                    chunk_global += sbnch

            # ---- layer 1 ----
            l1_tables = [hpre1[r * R1:(r + 1) * R1, :] for r in range(R)]

            with tc.tile_pool(name="mp_sb", bufs=2) as mp_sb, \
                 tc.tile_pool(name="mp_g", bufs=3) as mp_g, \
                 tc.tile_pool(name="mp_oh", bufs=2) as mp_oh, \
                 tc.tile_pool(name="blk", bufs=3) as blkp, \
                 tc.tile_pool(name="agg_ps", bufs=2, space="PSUM") as agg_ps, \
                 tc.tile_pool(name="tr_ps", bufs=2, space="PSUM") as tr_ps, \
                 tc.tile_pool(name="mm2_ps", bufs=2, space="PSUM") as mm2_ps:

                def epilogue1(sb, blocks, aggps):
                    ostage = blkp.tile([128, BPS, HID], BF, tag="ostage")
                    for bi, b in enumerate(blocks):
                        tmp = blkp.tile([128, HID], FP32, tag="tmp")
                        nc.vector.scalar_tensor_tensor(
                            out=tmp[:], in0=aggps[:, bi, :],
                            scalar=dinvbc[:, b:b + 1], in1=b1c[:],
                            op0=AOP.mult, op1=AOP.add)
                        h1b = blkp.tile([128, HID], FP32, tag="h1b")
                        nc.scalar.activation(out=h1b[:], in_=tmp[:], func=ACTF.Relu)
                        trp = tr_ps.tile([128, 128], FP32, space="PSUM")
                        nc.tensor.transpose(out=trp[:], in_=h1b[:],
                                            identity=ident[:])
                        h1t = blkp.tile([128, 128], FP32, tag="h1t")
                        nc.vector.tensor_copy(out=h1t[:], in_=trp[:])
                        mmp = mm2_ps.tile([128, HID], FP32, space="PSUM")
                        nc.tensor.matmul(mmp[:], h1t[:], w2c[:],
                                         start=True, stop=True)
                        nc.scalar.mul(out=ostage[:, bi, :], in_=mmp[:],
                                      mul=dinvbc[:, b:b + 1])
                    nb = len(blocks)
                    nc.sync.dma_start(
                        out=cc_in[sb * BPS * 128: sb * BPS * 128 + nb * 128, :]
                            .rearrange("(j p) f -> p j f", p=128),
                        in_=ostage[:, :nb, :])

                message_layer(1, sched1, idx1, dst1, l1_tables, epilogue1)

            if DEBUG:
                nc.sync.dma_start(out=dbg_hpre1[:], in_=hpre1[:])
                nc.sync.dma_start(out=dbg_ccin[:], in_=cc_in[:])

            # ---- all-gather hpre2' (4 range chunks) ----
            for r in range(R):
                nc.gpsimd.collective_compute(
                    "AllGather", AOP.bypass,
                    ins=[cc_in[r * R2:(r + 1) * R2, :]],
                    outs=[cc_out[r][:]],
                    replica_groups=[list(range(NCORES))])

            # ---- layer 2 ----
            l2_tables = [cc_out[r][:] for r in range(R)]

            with tc.tile_pool(name="mp_sb2", bufs=2) as mp_sb, \
                 tc.tile_pool(name="mp_g2", bufs=3) as mp_g, \
                 tc.tile_pool(name="mp_oh2", bufs=2) as mp_oh, \
                 tc.tile_pool(name="blk2", bufs=3) as blkp, \
                 tc.tile_pool(name="agg_ps2", bufs=2, space="PSUM") as agg_ps, \
                 tc.tile_pool(name="pool_ps", bufs=1, space="PSUM") as pool_psp:

                poolps = pool_psp.tile([G, HID], FP32, space="PSUM")

                def epilogue2(sb, blocks, aggps):
                    nb = len(blocks)
                    poh = blkp.tile([128, BPS, G], BF, tag="poh")
                    nc.sync.dma_start(
                        out=poh[:, :nb, :],
                        in_=pooloh[sb * BPS * 128: sb * BPS * 128 + nb * 128, :]
                            .rearrange("(j p) f -> p j f", p=128))
                    for bi, b in enumerate(blocks):
                        tmp = blkp.tile([128, HID], FP32, tag="tmp2")
                        nc.vector.scalar_tensor_tensor(
                            out=tmp[:], in0=aggps[:, bi, :],
                            scalar=dinvbc[:, b:b + 1], in1=b2c[:],
                            op0=AOP.mult, op1=AOP.add)
                        h2b = blkp.tile([128, HID], BF, tag="h2b")
                        nc.scalar.activation(out=h2b[:], in_=tmp[:], func=ACTF.Relu)
                        first = (sb == 0 and bi == 0)
                        last = (b == NB - 1)
                        nc.tensor.matmul(poolps[:], poh[:, bi, :], h2b[:],
                                         start=first, stop=last)

                message_layer(2, sched2, idx2, dst2, l2_tables, epilogue2)

                pooled = blkp.tile([G, HID], FP32, tag="pooled")
                nc.vector.tensor_copy(out=pooled[:], in_=poolps[:])
                nc.sync.dma_start(out=cc2_in[:], in_=pooled[:])
                if DEBUG:
                    nc.sync.dma_start(out=dbg_pooled[:], in_=pooled[:])

            # ---- all-reduce pooled sums + MLP ----
            nc.gpsimd.collective_compute(
                "AllReduce", AOP.add, ins=[cc2_in[:]], outs=[cc2_out[:]],
                replica_groups=[list(range(NCORES))])

            with tc.tile_pool(name="mlp", bufs=1) as mlpp, \
                 tc.tile_pool(name="mlp_ps", bufs=1, space="PSUM") as mlpps:
                pall = mlpp.tile([G, HID], FP32)
                nc.sync.dma_start(out=pall[:], in_=cc2_out[:])
                nc.vector.tensor_scalar_mul(pall[:], pall[:], invcntc[:, :1])
                ptp = mlpps.tile([128, G], FP32, space="PSUM")
                nc.tensor.transpose(out=ptp[:], in_=pall[:],
                                    identity=ident[:G, :G])
                pt = mlpp.tile([128, G], FP32)
                nc.vector.tensor_copy(out=pt[:], in_=ptp[:])
                l1ps = mlpps.tile([G, HID // 2], FP32, space="PSUM")
                nc.tensor.matmul(l1ps[:], pt[:], wl1c[:], start=True, stop=True)
                l1t = mlpp.tile([G, HID // 2], FP32)
                nc.vector.scalar_tensor_tensor(
                    out=l1t[:], in0=l1ps[:], scalar=1.0, in1=bl1c[:],
                    op0=AOP.mult, op1=AOP.add)
                l1r = mlpp.tile([G, HID // 2], FP32)
                nc.scalar.activation(out=l1r[:], in_=l1t[:], func=ACTF.Relu)
                l1tp = mlpps.tile([HID // 2, G], FP32, space="PSUM")
                nc.tensor.transpose(out=l1tp[:], in_=l1r[:],
                                    identity=ident[:G, :G])
                l1T = mlpp.tile([HID // 2, G], FP32)
                nc.vector.tensor_copy(out=l1T[:], in_=l1tp[:])
                l2ps = mlpps.tile([G, NCLS], FP32, space="PSUM")
                nc.tensor.matmul(l2ps[:], l1T[:], wl2c[:], start=True, stop=True)
                outt = mlpp.tile([G, NCLS], FP32)
                nc.vector.scalar_tensor_tensor(
                    out=outt[:], in0=l2ps[:], scalar=1.0, in1=bl2c[:],
                    op0=AOP.mult, op1=AOP.add)
                nc.sync.dma_start(out=out[:], in_=outt[:])

    nc.compile()
    return nc


def _get_program(sched1, sched2):
    if "nc" not in _CACHE:
        _CACHE["nc"] = _build_program(sched1, sched2)
    return _CACHE["nc"]


def run(inputs, trace=False, trace_kwargs=None):
    from concourse.bass_utils import run_bass_kernel_spmd

    sched1, sched2, in_maps = _preprocess(**inputs)
    nc = _get_program(sched1, sched2)
    kw = {}
    if trace:
        kw["trace"] = True
        if trace_kwargs:
            kw.update(trace_kwargs)
    res = run_bass_kernel_spmd(nc, in_maps, core_ids=list(range(NCORES)), **kw)
    return np.asarray(res.results[0]["out"]), res


def kernel(**inputs) -> np.ndarray:
    out, _ = run(inputs)
    return out


# revision 14
# speedup vs baseline: 1.7409x; 1.7409x over previous
"""GCN 3-layer classifier on 8 Trainium2 NeuronCores.

Strategy: partition dst nodes (and incident edges) across the 8 cores.
Each core:
  P0: computes hpre1' = (x @ W1) * dinv for ALL nodes (replicated; cheap on PE)
      stored as bf16 rows [Npad, 128] in its own DRAM.
  L1: message passing for its dst shard: edges grouped by (8-block
      super-block, src-range); dma_gather of source rows (int16 idx per
      25088-row range table), one-hot built on DVE (is_equal vs iota),
      segment-sum via PSUM-accumulated matmuls; per dst-block epilogue
      applies dinv/bias/relu and the W2 matmul, producing hpre2' rows.
  AllGather (4 range-chunks, overlapped) -> full hpre2' table per core.
  L2: same message passing; epilogue pools per-graph sums via one-hot
      matmul into a persistent PSUM accumulator.
  AllReduce pooled sums + on-device MLP -> [64, 10] logits.

Self-loops are extra edges (src=dst); the GCN normalization
norm_e = dinv[src]*dinv[dst] factorizes: dinv[src] is folded into the
gathered rows (hpre' pre-scaled), dinv[dst] applied per dst block.
"""

import sys

for _p in ("/opt/trn_rl_repo", "/root/.axon_site/_ro/trn_rl_repo"):
    if _p not in sys.path:
        sys.path.append(_p)

import numpy as np
import ml_dtypes

N = 100000
E = 1600000
G = 64
IN_DIM = 64
HID = 128
NCLS = 10

NCORES = 8
SH = 12544            # nodes per core shard (98 blocks of 128)
NPAD = SH * NCORES    # 100352
NB = 98               # dst blocks per core
BPS = 8               # blocks per super-block
NSB = 13              # super-blocks (12*8 + 2)
R = 4                 # src ranges (int16 gather tables < 32768 rows)
R1 = NPAD // R        # 25088: layer-1 range size (global contiguous)
R2 = SH // R          # 3136: layer-2 per-core slice size

BF16 = ml_dtypes.bfloat16

_CACHE = {}


def _blocks_of(sb):
    return list(range(sb * BPS, min(sb * BPS + BPS, NB)))


def _build_layer_schedule(all_src, all_dst, r_of_src, idx_of_src):
    """Shared chunk schedule + per-core padded slot arrays for one layer."""
    core = all_dst // SH
    b = (all_dst % SH) // 128
    dstloc = all_dst % 128
    r = r_of_src
    key = (core.astype(np.int64) * NB + b) * R + r
    counts = np.bincount(key, minlength=NCORES * NB * R).reshape(NCORES, NB, R)
    nch = np.maximum(0, -(-counts.max(axis=0) // 128))  # [NB, R] ceil/128 of max core

    # schedule order: (sb, r, b in sb)
    order = []
    for sb in range(NSB):
        for rr in range(R):
            for bb in _blocks_of(sb):
                order.append((bb, rr))
    ordpos = np.zeros((NB, R), np.int64)
    for i, (bb, rr) in enumerate(order):
        ordpos[bb, rr] = i
    nch_ord = np.array([nch[bb, rr] for (bb, rr) in order], np.int64)
    chunk_start_ord = np.concatenate([[0], np.cumsum(nch_ord)])[:-1]
    total_chunks = int(nch_ord.sum())
    slot_tot = total_chunks * 128

    group_slot_off = chunk_start_ord * 128  # per schedule-order group

    idx_tensors, dst_tensors = [], []
    for c in range(NCORES):
        sel = core == c
        k2 = ordpos[b[sel], r[sel]]
        perm = np.argsort(k2, kind="stable")
        k2s = k2[perm]
        grp_first = np.searchsorted(k2s, np.arange(len(order)))
        within = np.arange(len(k2s)) - grp_first[k2s]
        pos = group_slot_off[k2s] + within
        assert pos.max(initial=-1) < slot_tot

        idx_pad = np.zeros(slot_tot, np.int16)
        dst_pad = np.full(slot_tot, 200.0, np.float32)
        idx_pad[pos] = idx_of_src[sel][perm].astype(np.int16)
        dst_pad[pos] = dstloc[sel][perm]

        # wrapped int16 layout: slot i -> [16*g + i%16, i//16], replicated 8x
        wrapped = np.tile(idx_pad.reshape(-1, 16).T, (8, 1))
        idx_tensors.append(np.ascontiguousarray(wrapped))
        # dst layout: chunk j col j, partition = slot%128; bf16
        dst_tensors.append(np.ascontiguousarray(dst_pad.reshape(-1, 128).T.astype(BF16)))

    # per-block first/last chunk flags (r, k)
    blk_first, blk_last = {}, {}
    for bb in range(NB):
        rs = [rr for rr in range(R) if nch[bb, rr] > 0]
        assert rs, f"block {bb} has no chunks"
        blk_first[bb] = (rs[0], 0)
        blk_last[bb] = (rs[-1], int(nch[bb, rs[-1]]) - 1)

    return {
        "nch": nch,
        "total_chunks": total_chunks,
        "slot_tot": slot_tot,
        "idx": idx_tensors,
        "dst": dst_tensors,
        "blk_first": blk_first,
        "blk_last": blk_last,
    }


def _preprocess(x, src, dst, batch, W1, b1, W2, b2, Wl1, bl1, Wl2, bl2):
    src = np.asarray(src, np.int64)
    dst = np.asarray(dst, np.int64)
    batch = np.asarray(batch, np.int64)

    deg = np.bincount(dst, minlength=N).astype(np.float32) + 1.0
    dinv = 1.0 / np.sqrt(deg)
    dinv_pad = np.zeros(NPAD, np.float32)
    dinv_pad[:N] = dinv

    self_n = np.arange(N, dtype=np.int64)
    all_src = np.concatenate([src, self_n])
    all_dst = np.concatenate([dst, self_n])

    # layer 1 table: hpre1 rows, ranges = global contiguous quarters
    sched1 = _build_layer_schedule(all_src, all_dst, all_src // R1,
                                   all_src % R1)
    # layer 2 table: allgathered (core-major, shard-quarter) order
    c_of = all_src // SH
    l_of = all_src % SH
    sched2 = _build_layer_schedule(all_src, all_dst, l_of // R2,
                                   c_of * R2 + (l_of % R2))

    x_pad = np.zeros((NPAD, IN_DIM), np.float32)
    x_pad[:N] = np.asarray(x, np.float32)
    xT = np.ascontiguousarray(x_pad.T.astype(BF16))  # [64, NPAD]

    cnts = np.bincount(batch, minlength=G).astype(np.float32)
    invcnt = (1.0 / np.maximum(cnts, 1.0)).reshape(G, 1).astype(np.float32)

    common = {
        "xT": xT,
        "W1": np.asarray(W1, np.float32).astype(BF16),                # [64, 128]
        "W2": np.ascontiguousarray(np.asarray(W2, np.float32)),       # [128, 128]
        "Wl1": np.ascontiguousarray(np.asarray(Wl1, np.float32)),     # [128, 64]
        "Wl2": np.ascontiguousarray(np.asarray(Wl2, np.float32)),     # [64, 10]
        "b1b": np.tile(np.asarray(b1, np.float32)[None, :], (128, 1)),
        "b2b": np.tile(np.asarray(b2, np.float32)[None, :], (128, 1)),
        "bl1b": np.tile(np.asarray(bl1, np.float32)[None, :], (G, 1)),
        "bl2b": np.tile(np.asarray(bl2, np.float32)[None, :], (G, 1)),
        "dinv4": np.ascontiguousarray(dinv_pad.reshape(NPAD // 128, 128).T),
        "invcnt": invcnt,
    }

    in_maps = []
    for c in range(NCORES):
        lo = c * SH
        dinvb = np.ascontiguousarray(dinv_pad[lo:lo + SH].reshape(NB, 128).T)
        pooloh = np.zeros((SH, G), np.float32)
        hi_real = min(N, lo + SH)
        if hi_real > lo:
            loc = np.arange(hi_real - lo)
            pooloh[loc, batch[lo:hi_real]] = 1.0
        m = dict(common)
        m["dinvb"] = dinvb
        m["pooloh"] = np.ascontiguousarray(pooloh.astype(BF16))
        m["idx1"] = sched1["idx"][c]
        m["dst1"] = sched1["dst"][c]
        m["idx2"] = sched2["idx"][c]
        m["dst2"] = sched2["dst"][c]
        in_maps.append(m)

    return sched1, sched2, in_maps


DEBUG = False


def _build_program(sched1, sched2):
    import concourse.bass as bass
    import concourse.mybir as mybir
    import concourse.tile as tile
    from concourse import bacc
    from concourse.masks import make_identity

    FP32 = mybir.dt.float32
    BF = mybir.dt.bfloat16
    AOP = mybir.AluOpType
    ACTF = mybir.ActivationFunctionType

    nc = bacc.Bacc("TRN2", target_bir_lowering=False, debug=False,
                   num_devices=NCORES, num_swdge_queues=4)

    # ---- I/O -----------------------------------------------------------
    xT = nc.dram_tensor("xT", [IN_DIM, NPAD], BF, kind="ExternalInput")
    W1 = nc.dram_tensor("W1", [IN_DIM, HID], BF, kind="ExternalInput")
    W2 = nc.dram_tensor("W2", [HID, HID], FP32, kind="ExternalInput")
    Wl1 = nc.dram_tensor("Wl1", [HID, HID // 2], FP32, kind="ExternalInput")
    Wl2 = nc.dram_tensor("Wl2", [HID // 2, NCLS], FP32, kind="ExternalInput")
    b1b = nc.dram_tensor("b1b", [128, HID], FP32, kind="ExternalInput")
    b2b = nc.dram_tensor("b2b", [128, HID], FP32, kind="ExternalInput")
    bl1b = nc.dram_tensor("bl1b", [G, HID // 2], FP32, kind="ExternalInput")
    bl2b = nc.dram_tensor("bl2b", [G, NCLS], FP32, kind="ExternalInput")
    dinv4 = nc.dram_tensor("dinv4", [128, NPAD // 128], FP32, kind="ExternalInput")
    dinvb = nc.dram_tensor("dinvb", [128, NB], FP32, kind="ExternalInput")
    invcnt = nc.dram_tensor("invcnt", [G, 1], FP32, kind="ExternalInput")
    pooloh = nc.dram_tensor("pooloh", [SH, G], BF, kind="ExternalInput")
    idx1 = nc.dram_tensor("idx1", list(sched1["idx"][0].shape), mybir.dt.int16,
                          kind="ExternalInput")
    dst1 = nc.dram_tensor("dst1", list(sched1["dst"][0].shape), BF,
                          kind="ExternalInput")
    idx2 = nc.dram_tensor("idx2", list(sched2["idx"][0].shape), mybir.dt.int16,
                          kind="ExternalInput")
    dst2 = nc.dram_tensor("dst2", list(sched2["dst"][0].shape), BF,
                          kind="ExternalInput")
    out = nc.dram_tensor("out", [G, NCLS], FP32, kind="ExternalOutput")
    if DEBUG:
        dbg_hpre1 = nc.dram_tensor("dbg_hpre1", [NPAD, HID], BF,
                                   kind="ExternalOutput")
        dbg_ccin = nc.dram_tensor("dbg_ccin", [SH, HID], BF,
                                  kind="ExternalOutput")
        dbg_pooled = nc.dram_tensor("dbg_pooled", [G, HID], FP32,
                                    kind="ExternalOutput")

    # ---- internal DRAM -------------------------------------------------
    hpre1 = nc.dram_tensor("hpre1", [NPAD, HID], BF, kind="Internal")
    cc_in = nc.dram_tensor("cc_in", [SH, HID], BF, kind="Internal")
    cc_out = [
        nc.dram_tensor(f"cc_out{r}", [R1, HID], BF, kind="Internal",
                       addr_space="Shared")
        for r in range(R)
    ]
    cc2_in = nc.dram_tensor("cc2_in", [G, HID], FP32, kind="Internal")
    cc2_out = nc.dram_tensor("cc2_out", [G, HID], FP32, kind="Internal",
                             addr_space="Shared")

    max_call_nch = 0
    sb_nch = {}
    for sched in (sched1, sched2):
        for sb in range(NSB):
            blocks = _blocks_of(sb)
            tot = 0
            for r in range(R):
                nchr = int(sum(sched["nch"][b, r] for b in blocks))
                max_call_nch = max(max_call_nch, nchr)
                tot += nchr
            sb_nch[(id(sched), sb)] = tot
    max_sb_nch = max(
        sb_nch[(id(s), sb)] for s in (sched1, sched2) for sb in range(NSB)
    )

    with tile.TileContext(nc) as tc:
        with tc.tile_pool(name="const", bufs=1) as constp:
            w1c = constp.tile([IN_DIM, HID], BF)
            nc.sync.dma_start(out=w1c[:], in_=W1[:])
            w2c = constp.tile([HID, HID], FP32)
            nc.sync.dma_start(out=w2c[:], in_=W2[:])
            wl1c = constp.tile([HID, HID // 2], FP32)
            nc.sync.dma_start(out=wl1c[:], in_=Wl1[:])
            wl2c = constp.tile([HID // 2, NCLS], FP32)
            nc.sync.dma_start(out=wl2c[:], in_=Wl2[:])
            b1c = constp.tile([128, HID], FP32)
            nc.sync.dma_start(out=b1c[:], in_=b1b[:])
            b2c = constp.tile([128, HID], FP32)
            nc.sync.dma_start(out=b2c[:], in_=b2b[:])
            bl1c = constp.tile([G, HID // 2], FP32)
            nc.sync.dma_start(out=bl1c[:], in_=bl1b[:])
            bl2c = constp.tile([G, NCLS], FP32)
            nc.sync.dma_start(out=bl2c[:], in_=bl2b[:])
            dinv4c = constp.tile([128, NPAD // 128], FP32)
            nc.sync.dma_start(out=dinv4c[:], in_=dinv4[:])
            dinvbc = constp.tile([128, NB], FP32)
            nc.sync.dma_start(out=dinvbc[:], in_=dinvb[:])
            invcntc = constp.tile([G, 1], FP32)
            nc.sync.dma_start(out=invcntc[:], in_=invcnt[:])
            ident = constp.tile([128, 128], FP32)
            make_identity(nc, ident[:])
            iota_i = constp.tile([128, 128], mybir.dt.int16)
            nc.gpsimd.iota(iota_i[:], pattern=[[1, 128]], base=0,
                           channel_multiplier=0)
            iota_b = constp.tile([128, 128], BF)
            nc.vector.tensor_copy(out=iota_b[:], in_=iota_i[:])
            zc = constp.tile([128, 512], BF)
            nc.vector.memset(zc[:], 0)

            # ============ P0: hpre1' = (x @ W1) * dinv, all nodes ========
            NCH0 = NPAD // 128  # 784
            with tc.tile_pool(name="p0sb", bufs=3) as p0sb, \
                 tc.tile_pool(name="p0ps", bufs=2, space="PSUM") as p0ps:
                for g0 in range(0, NCH0, 8):
                    xt = p0sb.tile([IN_DIM, 8 * 128], BF, tag="xt")
                    nc.sync.dma_start(out=xt[:],
                                      in_=xT[:, g0 * 128:(g0 + 8) * 128])
                    ps = p0ps.tile([128, 8, HID], FP32, space="PSUM")
                    for j in range(8):
                        nc.tensor.matmul(ps[:, j, :],
                                         xt[:, j * 128:(j + 1) * 128],
                                         w1c[:], start=True, stop=True)
                    stage = p0sb.tile([128, 8, HID], BF, tag="stage")
                    nc.vector.tensor_tensor(
                        out=stage[:], in0=ps[:],
                        in1=dinv4c[:, g0:g0 + 8].unsqueeze(2)
                            .broadcast_to([128, 8, HID]),
                        op=AOP.mult)
                    nc.sync.dma_start(
                        out=hpre1[g0 * 128:(g0 + 8) * 128, :]
                            .rearrange("(j p) f -> p j f", p=128),
                        in_=stage[:])

            # ============ message-passing layers =========================
            def message_layer(layer, sched, idx_t, dst_t, tables, epilogue):
                nch = sched["nch"]
                chunk_global = 0
                qn = 0
                for sb in range(NSB):
                    blocks = _blocks_of(sb)
                    sbnch = int(sum(nch[b, r] for b in blocks for r in range(R)))
                    idxt = mp_sb.tile([128, max_sb_nch * 8], mybir.dt.int16,
                                      tag="idxt")
                    nc.sync.dma_start(
                        out=idxt[:, :sbnch * 8],
                        in_=idx_t[:, chunk_global * 8:(chunk_global + sbnch) * 8])
                    dstt = mp_sb.tile([128, max_sb_nch], BF, tag="dstt")
                    nc.sync.dma_start(
                        out=dstt[:, :sbnch],
                        in_=dst_t[:, chunk_global:chunk_global + sbnch])
                    aggps = agg_ps.tile([128, BPS, HID], FP32, space="PSUM")
                    # Zero-fill each PSUM bank with one start=True matmul.
                    # start clears has_written for the WHOLE bank, so the
                    # per-block accumulation groups below (which interleave
                    # within a bank across the range passes) must all use
                    # start=False on a pre-zeroed bank.
                    nc.tensor.matmul(aggps[:, 0:4, :], zc[:, :128], zc[:, :512],
                                     start=True, stop=True, skip_group_check=True)
                    nc.tensor.matmul(aggps[:, 4:8, :], zc[:, :128], zc[:, :512],
                                     start=True, stop=True, skip_group_check=True)
                    ch_in_sb = 0
                    for r in range(R):
                        nchr = int(sum(nch[b, r] for b in blocks))
                        if nchr == 0:
                            continue
                        gt = mp_g.tile([128, max_call_nch, HID], BF, tag="gt")
                        nc.gpsimd.dma_gather(
                            out_ap=gt[:, :nchr, :], in_ap=tables[r],
                            idxs_ap=idxt[:, ch_in_sb * 8:(ch_in_sb + nchr) * 8],
                            num_idxs=nchr * 128, num_idxs_reg=nchr * 128,
                            elem_size=HID, single_packet=False,
                            queue_num=qn % 4)
                        oht = mp_oh.tile([128, max_call_nch, 128], BF, tag="oht")
                        nc.vector.tensor_tensor(
                            out=oht[:, :nchr, :],
                            in0=iota_b[:].unsqueeze(1)
                                .broadcast_to([128, nchr, 128]),
                            in1=dstt[:, ch_in_sb:ch_in_sb + nchr].unsqueeze(2)
                                .broadcast_to([128, nchr, 128]),
                            op=AOP.is_equal)
                        j = 0
                        for bi, b in enumerate(blocks):
                            for k in range(int(nch[b, r])):
                                nc.tensor.matmul(
                                    aggps[:, bi, :], oht[:, j, :], gt[:, j, :],
                                    start=False,
                                    stop=(sched["blk_last"][b] == (r, k)),
                                    skip_group_check=True)
                                j += 1
                        ch_in_sb += nchr
                        qn += 1
                    epilogue(sb, blocks, aggps)
                    chunk_global += sbnch

            # ---- layer 1 ----
            l1_tables = [hpre1[r * R1:(r + 1) * R1, :] for r in range(R)]

            with tc.tile_pool(name="mp_sb", bufs=2) as mp_sb, \
                 tc.tile_pool(name="mp_g", bufs=5) as mp_g, \
                 tc.tile_pool(name="mp_oh", bufs=3) as mp_oh, \
                 tc.tile_pool(name="blk", bufs=3) as blkp, \
                 tc.tile_pool(name="agg_ps", bufs=2, space="PSUM") as agg_ps, \
                 tc.tile_pool(name="tr_ps", bufs=2, space="PSUM") as tr_ps, \
                 tc.tile_pool(name="mm2_ps", bufs=2, space="PSUM") as mm2_ps:

                def epilogue1(sb, blocks, aggps):
                    ostage = blkp.tile([128, BPS, HID], BF, tag="ostage")
                    for bi, b in enumerate(blocks):
                        tmp = blkp.tile([128, HID], FP32, tag="tmp")
                        nc.vector.scalar_tensor_tensor(
                            out=tmp[:], in0=aggps[:, bi, :],
                            scalar=dinvbc[:, b:b + 1], in1=b1c[:],
                            op0=AOP.mult, op1=AOP.add)
                        h1b = blkp.tile([128, HID], FP32, tag="h1b")
                        nc.scalar.activation(out=h1b[:], in_=tmp[:], func=ACTF.Relu)
                        trp = tr_ps.tile([128, 128], FP32, space="PSUM")
                        nc.tensor.transpose(out=trp[:], in_=h1b[:],
                                            identity=ident[:])
                        h1t = blkp.tile([128, 128], FP32, tag="h1t")
                        nc.vector.tensor_copy(out=h1t[:], in_=trp[:])
                        mmp = mm2_ps.tile([128, HID], FP32, space="PSUM")
                        nc.tensor.matmul(mmp[:], h1t[:], w2c[:],
                                         start=True, stop=True)
                        nc.scalar.mul(out=ostage[:, bi, :], in_=mmp[:],
                                      mul=dinvbc[:, b:b + 1])
                    nb = len(blocks)
                    nc.sync.dma_start(
                        out=cc_in[sb * BPS * 128: sb * BPS * 128 + nb * 128, :]
                            .rearrange("(j p) f -> p j f", p=128),
                        in_=ostage[:, :nb, :])

                message_layer(1, sched1, idx1, dst1, l1_tables, epilogue1)

            if DEBUG:
                nc.sync.dma_start(out=dbg_hpre1[:], in_=hpre1[:])
                nc.sync.dma_start(out=dbg_ccin[:], in_=cc_in[:])

            # ---- all-gather hpre2' (4 range chunks) ----
            for r in range(R):
                nc.gpsimd.collective_compute(
                    "AllGather", AOP.bypass,
                    ins=[cc_in[r * R2:(r + 1) * R2, :]],
                    outs=[cc_out[r][:]],
                    replica_groups=[list(range(NCORES))])

            # ---- layer 2 ----
            l2_tables = [cc_out[r][:] for r in range(R)]

            with tc.tile_pool(name="mp_sb2", bufs=2) as mp_sb, \
                 tc.tile_pool(name="mp_g2", bufs=5) as mp_g, \
                 tc.tile_pool(name="mp_oh2", bufs=3) as mp_oh, \
                 tc.tile_pool(name="blk2", bufs=3) as blkp, \
                 tc.tile_pool(name="agg_ps2", bufs=2, space="PSUM") as agg_ps, \
                 tc.tile_pool(name="pool_ps", bufs=1, space="PSUM") as pool_psp:

                poolps = pool_psp.tile([G, HID], FP32, space="PSUM")

                def epilogue2(sb, blocks, aggps):
                    nb = len(blocks)
                    poh = blkp.tile([128, BPS, G], BF, tag="poh")
                    nc.sync.dma_start(
                        out=poh[:, :nb, :],
                        in_=pooloh[sb * BPS * 128: sb * BPS * 128 + nb * 128, :]
                            .rearrange("(j p) f -> p j f", p=128))
                    for bi, b in enumerate(blocks):
                        tmp = blkp.tile([128, HID], FP32, tag="tmp2")
                        nc.vector.scalar_tensor_tensor(
                            out=tmp[:], in0=aggps[:, bi, :],
                            scalar=dinvbc[:, b:b + 1], in1=b2c[:],
                            op0=AOP.mult, op1=AOP.add)
                        h2b = blkp.tile([128, HID], BF, tag="h2b")
                        nc.scalar.activation(out=h2b[:], in_=tmp[:], func=ACTF.Relu)
                        first = (sb == 0 and bi == 0)
                        last = (b == NB - 1)
                        nc.tensor.matmul(poolps[:], poh[:, bi, :], h2b[:],
                                         start=first, stop=last)

                message_layer(2, sched2, idx2, dst2, l2_tables, epilogue2)

                pooled = blkp.tile([G, HID], FP32, tag="pooled")
                nc.vector.tensor_copy(out=pooled[:], in_=poolps[:])
                nc.sync.dma_start(out=cc2_in[:], in_=pooled[:])
                if DEBUG:
                    nc.sync.dma_start(out=dbg_pooled[:], in_=pooled[:])

            # ---- all-reduce pooled sums + MLP ----
            nc.gpsimd.collective_compute(
                "AllReduce", AOP.add, ins=[cc2_in[:]], outs=[cc2_out[:]],
                replica_groups=[list(range(NCORES))])

            with tc.tile_pool(name="mlp", bufs=1) as mlpp, \
                 tc.tile_pool(name="mlp_ps", bufs=1, space="PSUM") as mlpps:
                pall = mlpp.tile([G, HID], FP32)
                nc.sync.dma_start(out=pall[:], in_=cc2_out[:])
                nc.vector.tensor_scalar_mul(pall[:], pall[:], invcntc[:, :1])
                ptp = mlpps.tile([128, G], FP32, space="PSUM")
                nc.tensor.transpose(out=ptp[:], in_=pall[:],
                                    identity=ident[:G, :G])
                pt = mlpp.tile([128, G], FP32)
                nc.vector.tensor_copy(out=pt[:], in_=ptp[:])
                l1ps = mlpps.tile([G, HID // 2], FP32, space="PSUM")
                nc.tensor.matmul(l1ps[:], pt[:], wl1c[:], start=True, stop=True)
                l1t = mlpp.tile([G, HID // 2], FP32)
                nc.vector.scalar_tensor_tensor(
                    out=l1t[:], in0=l1ps[:], scalar=1.0, in1=bl1c[:],
                    op0=AOP.mult, op1=AOP.add)
                l1r = mlpp.tile([G, HID // 2], FP32)
                nc.scalar.activation(out=l1r[:], in_=l1t[:], func=ACTF.Relu)
                l1tp = mlpps.tile([HID // 2, G], FP32, space="PSUM")
                nc.tensor.transpose(out=l1tp[:], in_=l1r[:],
                                    identity=ident[:G, :G])
                l1T = mlpp.tile([HID // 2, G], FP32)
                nc.vector.tensor_copy(out=l1T[:], in_=l1tp[:])
                l2ps = mlpps.tile([G, NCLS], FP32, space="PSUM")
                nc.tensor.matmul(l2ps[:], l1T[:], wl2c[:], start=True, stop=True)
                outt = mlpp.tile([G, NCLS], FP32)
                nc.vector.scalar_tensor_tensor(
                    out=outt[:], in0=l2ps[:], scalar=1.0, in1=bl2c[:],
                    op0=AOP.mult, op1=AOP.add)
                nc.sync.dma_start(out=out[:], in_=outt[:])

    nc.compile()
    return nc


def _get_program(sched1, sched2):
    if "nc" not in _CACHE:
        _CACHE["nc"] = _build_program(sched1, sched2)
    return _CACHE["nc"]


def run(inputs, trace=False, trace_kwargs=None):
    from concourse.bass_utils import run_bass_kernel_spmd

    sched1, sched2, in_maps = _preprocess(**inputs)
    nc = _get_program(sched1, sched2)
    kw = {}
    if trace:
        kw["trace"] = True
        if trace_kwargs:
            kw.update(trace_kwargs)
    res = run_bass_kernel_spmd(nc, in_maps, core_ids=list(range(NCORES)), **kw)
    return np.asarray(res.results[0]["out"]), res


def kernel(**inputs) -> np.ndarray:
    out, _ = run(inputs)
    return out
